# revision 18
# baseline (speedup 1.0000x reference)
"""ConvChunk2d patch-extraction kernel for Trainium2 (8 NeuronCores).

Reference computes, for x of shape (8, 64, 128, 128):
    out[n, y*128 + xx, c, a, b] = xpad[n, (a*192 + b*64 + c) // 9, y + a, xx + b]
with xpad zero-padded by 1 on H/W, output shape (8*16384, 64, 3, 3).

Pure data movement (gather + replication), memory-bound.  Strategy:
data-parallel over batch (1 image per core).  Per core:
  - Load input ONCE as A0[y_partition, ch, x+1] (x zero-padded in the free
    dim), in ch-quarters so downstream work starts under the load.
  - Rows a=0 read input row y-1, rows a=2 read y+1; the partition-lockstep
    compute engines need those in partition y, so per x-block the idle
    TensorEngine matmuls A0 against a 0/1 shift-permutation matrix (extra
    kernel input) into PSUM, then Vector/Scalar copy to small SBUF halo
    tiles.  Key shrink: kernel row a=0 only ever reads channels
    ch = (64(0..2)+c)//9 <= 21 and a=2 only ch >= 42, so each halo holds
    just 22 channels (2.7x less TensorE/PSUM work than all 64).
  - Output assembly: out column j = c*9 + 3a+b reads channel
    ch(a,b,c) = (192a + 64b + c)//9.  For fixed a and phase
    phi = (192a + c) % 9 <= 6, the (c, b) pairs form an affine lattice:
    c = c0 + 9t, ch = ch0 + t + 7b, so ONE strided tensor_copy (custom AP
    with a 7*pitch+1 stride for the b axis) moves cnt*3*xb elements;
    phi in {7, 8} fall back to per-b copies.  39 logical families per
    x-block, balanced across Vector/Scalar/GPSIMD with trace-measured
    cost models (the greedy also chooses merged-vs-split per family).
  - Output tiles (128 rows y, xb*576 floats) DMA out as large contiguous
    runs per partition (xb*2304 bytes).  Small first/last blocks shorten
    the pipeline ramp and drain.
"""

import numpy as np

import concourse.bacc as bacc
import concourse.bass as bass
import concourse.mybir as mybir
from concourse.bass_utils import run_bass_kernel_spmd
from concourse.tile import TileContext

N, C, H, W = 8, 64, 128, 128
K = 3
L = H * W
J = C * K * K  # 576 output columns per spatial location
BLOCKS = [(0, 4), (4, 22), (26, 22), (48, 22), (70, 22), (92, 22), (114, 14)]
XBMAX = 22
HCH = 22  # channels held by each halo tile
F32 = mybir.dt.float32


def _jobs2():
    """Merged copy families.

    ("m", a, c0, cnt, ch0): dst T[:, :, c0+9t, 3a+b] <- src[ch0 + t + 7b]
        for t in [0,cnt), b in [0,3)  (one copy, custom b-stride AP)
    ("s", a, b, c0, cnt, ch): dst T[:, :, c0+9t, 3a+b] <- src[ch + t]
    """
    jobs = []
    for a in range(3):
        for phi in range(9):
            c0 = (phi - 192 * a) % 9
            cnt = (64 - c0 + 8) // 9
            ch0 = (192 * a + c0) // 9
            if phi <= 6:
                jobs.append(("m", a, c0, cnt, ch0))
            else:
                for b in range(3):
                    off = (phi + 64 * b) // 9
                    jobs.append(("s", a, b, c0, cnt, ch0 + off))
    return jobs


def build_nc():
    nc = bacc.Bacc("TRN2")
    x = nc.declare_dram_parameter("x", [C, H, W], F32, isOutput=False)
    sh = nc.declare_dram_parameter("sh", [128, 256], F32, isOutput=False)
    out = nc.declare_dram_parameter("out", [L, J], F32, isOutput=True)

    with TileContext(nc) as tc:
        with (
            tc.tile_pool(name="a", bufs=1) as apool,
            tc.tile_pool(name="h", bufs=2) as hpool,
            tc.tile_pool(name="t", bufs=3) as tpool,
            tc.tile_pool(name="ps", bufs=8, space="PSUM") as pspool,
        ):
            A0 = apool.tile([128, C, W + 2], F32, tag="a0")
            SH = apool.tile([128, 256], F32, tag="sh")
            nc.sync.dma_start(out=SH[:, :], in_=sh[:, :])

            # Zero-pad columns x=0 and x=W+1.
            nc.vector.memset(A0[:, :, 0:1], 0.0)
            nc.vector.memset(A0[:, :, W + 1 : W + 2], 0.0)
            # Load x[ch, y, xx] -> A0[y, ch, xx+1], split in ch-quarters on
            # FOUR different DMA queues: each SDMA engine then has multiple
            # outstanding 512B descriptors, hiding per-descriptor latency.
            qs = (nc.sync, nc.scalar, nc.gpsimd, nc.sync)
            for qi, cq in enumerate(range(0, C, 16)):
                qs[qi].dma_start(
                    out=A0[:, cq : cq + 16, 1 : W + 1],
                    in_=x[cq : cq + 16, :, :].transpose([1, 0, 2]),
                )

            jobs = _jobs2()
            outr = out[:, :].rearrange("(y xx) j -> y xx j", xx=W)
            # Greedy engine balancing with trace-measured per-copy cost
            # models (ns, e = elements per partition).  3-D single copies:
            #   V 95+3.15e / S 289+1.61e / G 205+3.15e; 4-D merged copies:
            #   V 601+1.80e / S 292+1.80e / G 190+3.71e (4-D APs carry a
            #   large fixed cost on DVE).  The greedy picks, per family,
            #   merged-on-one-engine vs split-into-3 by resulting makespan.
            for x0, xb in BLOCKS:
                # Balance per block: each block is one pipeline quantum, so
                # the block-local makespan is what gates the output DMA.
                load = [0.0, 0.0, 0.0]
                hw = xb + 2  # halo width in padded-x columns [x0, x0+hw)
                T = tpool.tile([128, XBMAX, C, K * K], F32, tag="t")
                Hm = hpool.tile([128, HCH, XBMAX + 2], F32, tag="hm")
                Hp = hpool.tile([128, HCH, XBMAX + 2], F32, tag="hp")
                # Row-shifted halos via TensorE: psum[m,:] = sum_k S[k,m] A0[k,:]
                # in 11-channel chunks (<=512 f32 = 1 PSUM bank each).
                for Hk, s0, cb in ((Hm, 0, 0), (Hp, 128, 42)):
                    for ch0, cn in ((0, 11), (11, 11)):
                        P = pspool.tile([128, 11 * (XBMAX + 2)], F32, tag="ps")
                        pc = P[:, : cn * hw].rearrange("y (c w) -> y c w", c=cn)
                        nc.tensor.matmul(
                            pc,
                            SH[:, s0 : s0 + 128],
                            A0[:, cb + ch0 : cb + ch0 + cn, x0 : x0 + hw],
                        )
                        e = cn * hw
                        cv, cs = 147 + 1.03 * e, 254 + 0.84 * e
                        if load[0] + cv <= load[1] + cs:
                            nc.vector.tensor_copy(Hk[:, ch0 : ch0 + cn, 0:hw], pc)
                            load[0] += cv
                        else:
                            nc.scalar.copy(Hk[:, ch0 : ch0 + cn, 0:hw], pc)
                            load[1] += cs

                engines = (nc.vector.tensor_copy, nc.scalar.copy, nc.gpsimd.tensor_copy)

                def emit(dst, src, costs):
                    eng = min(range(3), key=lambda i: load[i] + costs[i])
                    load[eng] += costs[eng]
                    engines[eng](dst, src)

                def single_aps(Sk, xc, rw, a, b, c0, cnt):
                    dst = T[:, :xb, c0 : c0 + 9 * (cnt - 1) + 1 : 9, 3 * a + b]
                    src = Sk[:, rw : rw + cnt, xc + b : xc + b + xb].transpose(
                        [0, 2, 1]
                    )
                    return dst, src

                for job in jobs:
                    a = job[1]
                    if a == 1:
                        Sk, pitch, xc, cb = A0, W + 2, x0, 0
                    else:
                        Sk = Hm if a == 0 else Hp
                        pitch, xc, cb = XBMAX + 2, 0, (0 if a == 0 else 42)
                    if job[0] == "m":
                        _, a, c0, cnt, ch0 = job
                        em, es = 3 * cnt * xb, cnt * xb
                        cm = (601 + 1.80 * em, 292 + 1.80 * em, 190 + 3.71 * em)
                        csc = (95 + 3.15 * es, 289 + 1.61 * es, 205 + 3.15 * es)
                        # merged on one engine vs three singles, by makespan
                        lm = list(load)
                        im = min(range(3), key=lambda i: lm[i] + cm[i])
                        lm[im] += cm[im]
                        ls = list(load)
                        for _b in range(3):
                            i = min(range(3), key=lambda j: ls[j] + csc[j])
                            ls[i] += csc[i]
                        if (max(lm), sum(lm)) <= (max(ls), sum(ls)):
                            dst = T[
                                :, :xb, c0 : c0 + 9 * (cnt - 1) + 1 : 9, 3 * a : 3 * a + 3
                            ]
                            src = (
                                Sk[:, ch0 - cb : ch0 - cb + cnt, xc : xc + xb]
                                .transpose([0, 2, 1])
                                .unsqueeze(3)
                            )
                            src.ap[3] = [7 * pitch + 1, 3]
                            load[im] += cm[im]
                            engines[im](dst, src)
                        else:
                            for b in range(3):
                                dst, src = single_aps(
                                    Sk, xc, ch0 - cb + 7 * b, a, b, c0, cnt
                                )
                                emit(dst, src, csc)
                    else:
                        _, a, b, c0, cnt, ch = job
                        dst, src = single_aps(Sk, xc, ch - cb, a, b, c0, cnt)
                        es = cnt * xb
                        emit(dst, src, (95 + 3.15 * es, 289 + 1.61 * es, 205 + 3.15 * es))
                nc.sync.dma_start(
                    out=outr[:, x0 : x0 + xb, :],
                    in_=T[:, :xb, :, :].rearrange("pp xx c q -> pp xx (c q)"),
                )
    nc.finalize()
    return nc


def _shift_mats():
    s = np.zeros((128, 256), dtype=np.float32)
    s[:, 0:128] = np.eye(128, k=1, dtype=np.float32)  # S_m: out[y] = in[y-1]
    s[:, 128:256] = np.eye(128, k=-1, dtype=np.float32)  # S_p: out[y] = in[y+1]
    return s


def make_in_maps(x):
    s = _shift_mats()
    return [{"x": x[n], "sh": s} for n in range(N)]


def kernel(x):
    x = np.ascontiguousarray(np.asarray(x, dtype=np.float32))
    assert x.shape == (N, C, H, W), x.shape
    nc = build_nc()
    in_maps = make_in_maps(x)
    res = run_bass_kernel_spmd(nc, in_maps, list(range(N)))
    outs = [np.asarray(res.results[i]["out"]).reshape(L, C, K, K) for i in range(N)]
    return np.concatenate(outs, axis=0)


# revision 20
# speedup vs baseline: 1.1441x; 1.1441x over previous
"""ConvChunk2d patch-extraction kernel for Trainium2 (8 NeuronCores).

Reference computes, for x of shape (8, 64, 128, 128):
    out[n, y*128 + xx, c, a, b] = xpad[n, (a*192 + b*64 + c) // 9, y + a, xx + b]
with xpad zero-padded by 1 on H/W, output shape (8*16384, 64, 3, 3).

Pure data movement (gather + replication), memory-bound.  Strategy:
data-parallel over batch (1 image per core).  Per core:
  - Load input ONCE as A0[y_partition, ch, x+1] (x zero-padded in the free
    dim), in ch-quarters so downstream work starts under the load.
  - Rows a=0 read input row y-1, rows a=2 read y+1; the partition-lockstep
    compute engines need those in partition y, so per x-block the idle
    TensorEngine matmuls A0 against a 0/1 shift-permutation matrix (extra
    kernel input) into PSUM, then Vector/Scalar copy to small SBUF halo
    tiles.  Key shrink: kernel row a=0 only ever reads channels
    ch = (64(0..2)+c)//9 <= 21 and a=2 only ch >= 42, so each halo holds
    just 22 channels (2.7x less TensorE/PSUM work than all 64).
  - Output assembly: out column j = c*9 + 3a+b reads channel
    ch(a,b,c) = (192a + 64b + c)//9.  For fixed a and phase
    phi = (192a + c) % 9 <= 6, the (c, b) pairs form an affine lattice:
    c = c0 + 9t, ch = ch0 + t + 7b, so ONE strided tensor_copy (custom AP
    with a 7*pitch+1 stride for the b axis) moves cnt*3*xb elements;
    phi in {7, 8} fall back to per-b copies.  39 logical families per
    x-block, balanced across Vector/Scalar/GPSIMD with trace-measured
    cost models (the greedy also chooses merged-vs-split per family).
  - Output tiles (128 rows y, xb*576 floats) DMA out as large contiguous
    runs per partition (xb*2304 bytes).  Small first/last blocks shorten
    the pipeline ramp and drain.
"""

import numpy as np

import concourse.bacc as bacc
import concourse.bass as bass
import concourse.mybir as mybir
from concourse.bass_utils import run_bass_kernel_spmd
from concourse.tile import TileContext

N, C, H, W = 8, 64, 128, 128
K = 3
L = H * W
J = C * K * K  # 576 output columns per spatial location
BLOCKS = [(0, 12), (12, 22), (34, 22), (56, 22), (78, 22), (100, 22), (122, 6)]
XBMAX = 22
HCH = 22  # channels held by each halo tile
F32 = mybir.dt.float32


def _jobs2():
    """Merged copy families.

    ("m", a, c0, cnt, ch0): dst T[:, :, c0+9t, 3a+b] <- src[ch0 + t + 7b]
        for t in [0,cnt), b in [0,3)  (one copy, custom b-stride AP)
    ("s", a, b, c0, cnt, ch): dst T[:, :, c0+9t, 3a+b] <- src[ch + t]
    """
    jobs = []
    for a in range(3):
        for phi in range(9):
            c0 = (phi - 192 * a) % 9
            cnt = (64 - c0 + 8) // 9
            ch0 = (192 * a + c0) // 9
            if phi <= 6:
                jobs.append(("m", a, c0, cnt, ch0))
            else:
                for b in range(3):
                    off = (phi + 64 * b) // 9
                    jobs.append(("s", a, b, c0, cnt, ch0 + off))
    return jobs


def build_nc():
    nc = bacc.Bacc("TRN2")
    x = nc.declare_dram_parameter("x", [C, H, W], F32, isOutput=False)
    sh = nc.declare_dram_parameter("sh", [128, 256], F32, isOutput=False)
    out = nc.declare_dram_parameter("out", [L, J], F32, isOutput=True)

    with TileContext(nc) as tc:
        with (
            tc.tile_pool(name="a", bufs=1) as apool,
            tc.tile_pool(name="h", bufs=2) as hpool,
            tc.tile_pool(name="t", bufs=3) as tpool,
            tc.tile_pool(name="ps", bufs=8, space="PSUM") as pspool,
        ):
            A0 = apool.tile([128, C, W + 2], F32, tag="a0")
            SH = apool.tile([128, 256], F32, tag="sh")
            nc.sync.dma_start(out=SH[:, :], in_=sh[:, :])

            # Zero-pad columns x=0 and x=W+1.
            nc.vector.memset(A0[:, :, 0:1], 0.0)
            nc.vector.memset(A0[:, :, W + 1 : W + 2], 0.0)
            # Load x[ch, y, xx] -> A0[y, ch, xx+1], split in ch-quarters so
            # the first shift-matmuls and copies can start under the load.
            for cq in range(0, C, 16):
                nc.sync.dma_start(
                    out=A0[:, cq : cq + 16, 1 : W + 1],
                    in_=x[cq : cq + 16, :, :].transpose([1, 0, 2]),
                )

            jobs = _jobs2()
            outr = out[:, :].rearrange("(y xx) j -> y xx j", xx=W)
            # Greedy engine balancing with trace-measured per-copy cost
            # models (ns, e = elements per partition).  3-D single copies:
            #   V 95+3.15e / S 289+1.61e / G 205+3.15e; 4-D merged copies:
            #   V 601+1.80e / S 292+1.80e / G 190+3.71e (4-D APs carry a
            #   large fixed cost on DVE).  The greedy picks, per family,
            #   merged-on-one-engine vs split-into-3 by resulting makespan.
            for x0, xb in BLOCKS:
                # Balance per block: each block is one pipeline quantum, so
                # the block-local makespan is what gates the output DMA.
                load = [0.0, 0.0, 0.0]
                hw = xb + 2  # halo width in padded-x columns [x0, x0+hw)
                T = tpool.tile([128, XBMAX, C, K * K], F32, tag="t")
                Hm = hpool.tile([128, HCH, XBMAX + 2], F32, tag="hm")
                Hp = hpool.tile([128, HCH, XBMAX + 2], F32, tag="hp")
                # Row-shifted halos via TensorE: psum[m,:] = sum_k S[k,m] A0[k,:]
                # in 11-channel chunks (<=512 f32 = 1 PSUM bank each).
                for Hk, s0, cb in ((Hm, 0, 0), (Hp, 128, 42)):
                    for ch0, cn in ((0, 11), (11, 11)):
                        P = pspool.tile([128, 11 * (XBMAX + 2)], F32, tag="ps")
                        pc = P[:, : cn * hw].rearrange("y (c w) -> y c w", c=cn)
                        nc.tensor.matmul(
                            pc,
                            SH[:, s0 : s0 + 128],
                            A0[:, cb + ch0 : cb + ch0 + cn, x0 : x0 + hw],
                        )
                        e = cn * hw
                        cv, cs = 147 + 1.03 * e, 254 + 0.84 * e
                        if load[0] + cv <= load[1] + cs:
                            nc.vector.tensor_copy(Hk[:, ch0 : ch0 + cn, 0:hw], pc)
                            load[0] += cv
                        else:
                            nc.scalar.copy(Hk[:, ch0 : ch0 + cn, 0:hw], pc)
                            load[1] += cs

                engines = (nc.vector.tensor_copy, nc.scalar.copy, nc.gpsimd.tensor_copy)

                def emit(dst, src, costs):
                    eng = min(range(3), key=lambda i: load[i] + costs[i])
                    load[eng] += costs[eng]
                    engines[eng](dst, src)

                def single_aps(Sk, xc, rw, a, b, c0, cnt):
                    dst = T[:, :xb, c0 : c0 + 9 * (cnt - 1) + 1 : 9, 3 * a + b]
                    src = Sk[:, rw : rw + cnt, xc + b : xc + b + xb].transpose(
                        [0, 2, 1]
                    )
                    return dst, src

                for job in jobs:
                    a = job[1]
                    if a == 1:
                        Sk, pitch, xc, cb = A0, W + 2, x0, 0
                    else:
                        Sk = Hm if a == 0 else Hp
                        pitch, xc, cb = XBMAX + 2, 0, (0 if a == 0 else 42)
                    if job[0] == "m":
                        _, a, c0, cnt, ch0 = job
                        em, es = 3 * cnt * xb, cnt * xb
                        cm = (601 + 1.80 * em, 292 + 1.80 * em, 190 + 3.71 * em)
                        csc = (95 + 3.15 * es, 289 + 1.61 * es, 205 + 3.15 * es)
                        # merged on one engine vs three singles, by makespan
                        lm = list(load)
                        im = min(range(3), key=lambda i: lm[i] + cm[i])
                        lm[im] += cm[im]
                        ls = list(load)
                        for _b in range(3):
                            i = min(range(3), key=lambda j: ls[j] + csc[j])
                            ls[i] += csc[i]
                        if (max(lm), sum(lm)) <= (max(ls), sum(ls)):
                            dst = T[
                                :, :xb, c0 : c0 + 9 * (cnt - 1) + 1 : 9, 3 * a : 3 * a + 3
                            ]
                            src = (
                                Sk[:, ch0 - cb : ch0 - cb + cnt, xc : xc + xb]
                                .transpose([0, 2, 1])
                                .unsqueeze(3)
                            )
                            src.ap[3] = [7 * pitch + 1, 3]
                            load[im] += cm[im]
                            engines[im](dst, src)
                        else:
                            for b in range(3):
                                dst, src = single_aps(
                                    Sk, xc, ch0 - cb + 7 * b, a, b, c0, cnt
                                )
                                emit(dst, src, csc)
                    else:
                        _, a, b, c0, cnt, ch = job
                        dst, src = single_aps(Sk, xc, ch - cb, a, b, c0, cnt)
                        es = cnt * xb
                        emit(dst, src, (95 + 3.15 * es, 289 + 1.61 * es, 205 + 3.15 * es))
                nc.sync.dma_start(
                    out=outr[:, x0 : x0 + xb, :],
                    in_=T[:, :xb, :, :].rearrange("pp xx c q -> pp xx (c q)"),
                )
    nc.finalize()
    return nc


def _shift_mats():
    s = np.zeros((128, 256), dtype=np.float32)
    s[:, 0:128] = np.eye(128, k=1, dtype=np.float32)  # S_m: out[y] = in[y-1]
    s[:, 128:256] = np.eye(128, k=-1, dtype=np.float32)  # S_p: out[y] = in[y+1]
    return s


def make_in_maps(x):
    s = _shift_mats()
    return [{"x": x[n], "sh": s} for n in range(N)]


def kernel(x):
    x = np.ascontiguousarray(np.asarray(x, dtype=np.float32))
    assert x.shape == (N, C, H, W), x.shape
    nc = build_nc()
    in_maps = make_in_maps(x)
    res = run_bass_kernel_spmd(nc, in_maps, list(range(N)))
    outs = [np.asarray(res.results[i]["out"]).reshape(L, C, K, K) for i in range(N)]
    return np.concatenate(outs, axis=0)


# revision 23
# speedup vs baseline: 1.1454x; 1.0012x over previous
"""ConvChunk2d patch-extraction kernel for Trainium2 (8 NeuronCores).

Reference computes, for x of shape (8, 64, 128, 128):
    out[n, y*128 + xx, c, a, b] = xpad[n, (a*192 + b*64 + c) // 9, y + a, xx + b]
with xpad zero-padded by 1 on H/W, output shape (8*16384, 64, 3, 3).

Pure data movement (gather + replication), memory-bound.  Strategy:
data-parallel over batch (1 image per core).  Per core:
  - Load input ONCE as A0[y_partition, ch, x+1] (x zero-padded in the free
    dim), in ch-quarters so downstream work starts under the load.
  - Rows a=0 read input row y-1, rows a=2 read y+1; the partition-lockstep
    compute engines need those in partition y, so per x-block the idle
    TensorEngine matmuls A0 against a 0/1 shift-permutation matrix (extra
    kernel input) into PSUM, then Vector/Scalar copy to small SBUF halo
    tiles.  Key shrink: kernel row a=0 only ever reads channels
    ch = (64(0..2)+c)//9 <= 21 and a=2 only ch >= 42, so each halo holds
    just 22 channels (2.7x less TensorE/PSUM work than all 64).
  - Output assembly: out column j = c*9 + 3a+b reads channel
    ch(a,b,c) = (192a + 64b + c)//9.  For fixed a and phase
    phi = (192a + c) % 9 <= 6, the (c, b) pairs form an affine lattice:
    c = c0 + 9t, ch = ch0 + t + 7b, so ONE strided tensor_copy (custom AP
    with a 7*pitch+1 stride for the b axis) moves cnt*3*xb elements;
    phi in {7, 8} fall back to per-b copies.  39 logical families per
    x-block, balanced across Vector/Scalar/GPSIMD with trace-measured
    cost models (the greedy also chooses merged-vs-split per family).
  - Output tiles (128 rows y, xb*576 floats) DMA out as large contiguous
    runs per partition (xb*2304 bytes).  Small first/last blocks shorten
    the pipeline ramp and drain.
"""

import numpy as np

import concourse.bacc as bacc
import concourse.bass as bass
import concourse.mybir as mybir
from concourse.bass_utils import run_bass_kernel_spmd
from concourse.tile import TileContext

N, C, H, W = 8, 64, 128, 128
K = 3
L = H * W
J = C * K * K  # 576 output columns per spatial location
BLOCKS = [(0, 6), (6, 16), (22, 22), (44, 22), (66, 22), (88, 22), (110, 18)]
XBMAX = 22
HCH = 22  # channels held by each halo tile
F32 = mybir.dt.float32


def _jobs2():
    """Merged copy families.

    ("m", a, c0, cnt, ch0): dst T[:, :, c0+9t, 3a+b] <- src[ch0 + t + 7b]
        for t in [0,cnt), b in [0,3)  (one copy, custom b-stride AP)
    ("s", a, b, c0, cnt, ch): dst T[:, :, c0+9t, 3a+b] <- src[ch + t]
    """
    jobs = []
    for a in range(3):
        for phi in range(9):
            c0 = (phi - 192 * a) % 9
            cnt = (64 - c0 + 8) // 9
            ch0 = (192 * a + c0) // 9
            if phi <= 6:
                jobs.append(("m", a, c0, cnt, ch0))
            else:
                for b in range(3):
                    off = (phi + 64 * b) // 9
                    jobs.append(("s", a, b, c0, cnt, ch0 + off))
    return jobs


def build_nc():
    nc = bacc.Bacc("TRN2")
    x = nc.declare_dram_parameter("x", [C, H, W], F32, isOutput=False)
    sh = nc.declare_dram_parameter("sh", [128, 256], F32, isOutput=False)
    out = nc.declare_dram_parameter("out", [L, J], F32, isOutput=True)

    with TileContext(nc) as tc:
        with (
            tc.tile_pool(name="a", bufs=1) as apool,
            tc.tile_pool(name="h", bufs=3) as hpool,
            tc.tile_pool(name="t", bufs=3) as tpool,
            tc.tile_pool(name="ps", bufs=8, space="PSUM") as pspool,
        ):
            A0 = apool.tile([128, C, W + 2], F32, tag="a0")
            SH = apool.tile([128, 256], F32, tag="sh")
            nc.sync.dma_start(out=SH[:, :], in_=sh[:, :])

            # Zero-pad columns x=0 and x=W+1.
            nc.vector.memset(A0[:, :, 0:1], 0.0)
            nc.vector.memset(A0[:, :, W + 1 : W + 2], 0.0)
            # Load x[ch, y, xx] -> A0[y, ch, xx+1], split in ch-quarters so
            # the first shift-matmuls and copies can start under the load.
            for cq in range(0, C, 16):
                nc.sync.dma_start(
                    out=A0[:, cq : cq + 16, 1 : W + 1],
                    in_=x[cq : cq + 16, :, :].transpose([1, 0, 2]),
                )

            jobs = _jobs2()
            outr = out[:, :].rearrange("(y xx) j -> y xx j", xx=W)
            # Greedy engine balancing with trace-measured per-copy cost
            # models (ns, e = elements per partition).  3-D single copies:
            #   V 95+3.15e / S 289+1.61e / G 205+3.15e; 4-D merged copies:
            #   V 601+1.80e / S 292+1.80e / G 190+3.71e (4-D APs carry a
            #   large fixed cost on DVE).  The greedy picks, per family,
            #   merged-on-one-engine vs split-into-3 by resulting makespan.
            engines = (nc.vector.tensor_copy, nc.scalar.copy, nc.gpsimd.tensor_copy)
            load = [0.0, 0.0, 0.0]

            def halo_production(x0, xb):
                """Emit shift-matmuls + PSUM->SBUF halo copies for one block.

                Returns (Hm, Hp).  Called AHEAD of the fams of earlier blocks
                (software pipelining): engines execute their streams in
                order, so halo copies must sit in the Vector/Scalar FIFOs
                BEFORE older blocks' fams or halo production can never run
                ahead of consumption.
                """
                hw = xb + 2
                Hm = hpool.tile([128, HCH, XBMAX + 2], F32, tag="hm", name="Hm")
                Hp = hpool.tile([128, HCH, XBMAX + 2], F32, tag="hp", name="Hp")
                for Hk, s0, cb in ((Hm, 0, 0), (Hp, 128, 42)):
                    for ch0, cn in ((0, 11), (11, 11)):
                        P = pspool.tile(
                            [128, 11 * (XBMAX + 2)], F32, tag="ps", name="P"
                        )
                        pc = P[:, : cn * hw].rearrange("y (c w) -> y c w", c=cn)
                        nc.tensor.matmul(
                            pc,
                            SH[:, s0 : s0 + 128],
                            A0[:, cb + ch0 : cb + ch0 + cn, x0 : x0 + hw],
                        )
                        e = cn * hw
                        cv, cs = 147 + 1.03 * e, 254 + 0.84 * e
                        if load[0] + cv <= load[1] + cs:
                            nc.vector.tensor_copy(Hk[:, ch0 : ch0 + cn, 0:hw], pc)
                            load[0] += cv
                        else:
                            nc.scalar.copy(Hk[:, ch0 : ch0 + cn, 0:hw], pc)
                            load[1] += cs
                return Hm, Hp

            def emit(dst, src, costs):
                eng = min(range(3), key=lambda i: load[i] + costs[i])
                load[eng] += costs[eng]
                engines[eng](dst, src)

            def fams_and_dma(x0, xb, Hm, Hp):
                """Emit the 39 copy families + the output DMA for one block."""

                T = tpool.tile([128, XBMAX, C, K * K], F32, tag="t", name="T")

                def single_aps(Sk, xc, rw, a, b, c0, cnt):
                    dst = T[:, :xb, c0 : c0 + 9 * (cnt - 1) + 1 : 9, 3 * a + b]
                    src = Sk[:, rw : rw + cnt, xc + b : xc + b + xb].transpose(
                        [0, 2, 1]
                    )
                    return dst, src

                for job in jobs:
                    a = job[1]
                    if a == 1:
                        Sk, pitch, xc, cb = A0, W + 2, x0, 0
                    else:
                        Sk = Hm if a == 0 else Hp
                        pitch, xc, cb = XBMAX + 2, 0, (0 if a == 0 else 42)
                    if job[0] == "m":
                        _, a, c0, cnt, ch0 = job
                        em, es = 3 * cnt * xb, cnt * xb
                        cm = (601 + 1.80 * em, 292 + 1.80 * em, 190 + 3.71 * em)
                        csc = (95 + 3.15 * es, 289 + 1.61 * es, 205 + 3.15 * es)
                        # merged on one engine vs three singles, by makespan
                        lm = list(load)
                        im = min(range(3), key=lambda i: lm[i] + cm[i])
                        lm[im] += cm[im]
                        ls = list(load)
                        for _b in range(3):
                            i = min(range(3), key=lambda j: ls[j] + csc[j])
                            ls[i] += csc[i]
                        if (max(lm), sum(lm)) <= (max(ls), sum(ls)):
                            dst = T[
                                :, :xb, c0 : c0 + 9 * (cnt - 1) + 1 : 9, 3 * a : 3 * a + 3
                            ]
                            src = (
                                Sk[:, ch0 - cb : ch0 - cb + cnt, xc : xc + xb]
                                .transpose([0, 2, 1])
                                .unsqueeze(3)
                            )
                            src.ap[3] = [7 * pitch + 1, 3]
                            load[im] += cm[im]
                            engines[im](dst, src)
                        else:
                            for b in range(3):
                                dst, src = single_aps(
                                    Sk, xc, ch0 - cb + 7 * b, a, b, c0, cnt
                                )
                                emit(dst, src, csc)
                    else:
                        _, a, b, c0, cnt, ch = job
                        dst, src = single_aps(Sk, xc, ch - cb, a, b, c0, cnt)
                        es = cnt * xb
                        emit(
                            dst,
                            src,
                            (95 + 3.15 * es, 289 + 1.61 * es, 205 + 3.15 * es),
                        )
                nc.sync.dma_start(
                    out=outr[:, x0 : x0 + xb, :],
                    in_=T[:, :xb, :, :].rearrange("pp xx c q -> pp xx (c q)"),
                )

            # Software pipeline: halo production runs AHEAD (depth bounded by
            # halo bufs=3 and psum bufs=8 = 2 blocks of chunks).
            halos = [halo_production(x0, xb) for x0, xb in BLOCKS[:3]]
            for k, (x0, xb) in enumerate(BLOCKS):
                fams_and_dma(x0, xb, *halos[k])
                if k + 3 < len(BLOCKS):
                    halos.append(halo_production(*BLOCKS[k + 3]))
    nc.finalize()
    return nc


def _shift_mats():
    s = np.zeros((128, 256), dtype=np.float32)
    s[:, 0:128] = np.eye(128, k=1, dtype=np.float32)  # S_m: out[y] = in[y-1]
    s[:, 128:256] = np.eye(128, k=-1, dtype=np.float32)  # S_p: out[y] = in[y+1]
    return s


def make_in_maps(x):
    s = _shift_mats()
    return [{"x": x[n], "sh": s} for n in range(N)]


def kernel(x):
    x = np.ascontiguousarray(np.asarray(x, dtype=np.float32))
    assert x.shape == (N, C, H, W), x.shape
    nc = build_nc()
    in_maps = make_in_maps(x)
    res = run_bass_kernel_spmd(nc, in_maps, list(range(N)))
    outs = [np.asarray(res.results[i]["out"]).reshape(L, C, K, K) for i in range(N)]
    return np.concatenate(outs, axis=0)
